# revision 24
# baseline (speedup 1.0000x reference)
"""Trainium2 Bass kernel for batched min-distance retrieval (KNN, K=1).

Computes, for embeds [16,4096,512] and centroids [2048,512]:
    score[b,n] = min_c sqrt(||e_bn||^2 + ||c||^2 - 2 e_bn.c)   -> [16,1,64,64]
    loss = (1/NU) * mean(relu(score - r^2))

Sharding: data-parallel over batch B across 8 cores (2 batches/core);
centroid bank replicated; loss partial-sums combined on host.

Per core, per 128-query chunk:
  - embeds are cast-DMA'd to bf16 and PE-transposed to put the contraction
    dim (d) on partitions,
  - bf16 matmuls compute -2 e.c into PSUM [128q x 512c]; each PSUM
    accumulation group is initialized by a K=3 matmul that reconstructs
    ||c||^2 as bf16(cb-512) + bf16(residual) + 512 (keeps the constant
    term at ~1e-4 absolute error despite bf16 operands),
  - DVE min-reduces each PSUM chunk; ScalarE computes ||e||^2
    (Square+accumulate) and the final sqrt(min + ||e||^2).
Loss: ScalarE Relu+accumulate over all scores, partition-reduced with a
tiny fp32 matmul against ones; host combines the 8 per-core partials.
"""

import numpy as np
from contextlib import ExitStack

import concourse.bass as bass
import concourse.tile as tile
import concourse.mybir as mybir
from concourse import bacc
from concourse.bass_utils import run_bass_kernel_spmd
from concourse.masks import make_identity

F32 = mybir.dt.float32
F32R = mybir.dt.float32r
BF16 = mybir.dt.bfloat16
AF = mybir.ActivationFunctionType
ALU = mybir.AluOpType

B, N, D, C = 16, 4096, 512, 2048
NU = 0.001
NCORES = 8
BLOC = B // NCORES            # batches per core
NQ = BLOC * N                 # 8192 queries per core
NCH = NQ // 128               # 64 query chunks of 128
DC = D // 128                 # 4 contraction chunks
CW = 512                      # centroid tile width (one PSUM bank of fp32)
NCC = C // CW                 # 4 centroid chunks
CB_SHIFT = 512.0              # recenters ||c||^2 (E[||c||^2] = D) for bf16

_PROG = None
LAST_RESULTS = None
RUN_KWARGS = {}  # test-harness hook (e.g. trace=True); empty in production


def _build_program(mm_dtype="bf16"):
    # Bacc (not raw Bass): its compile() pipeline splits multi-wait sync
    # conditions into event semaphores, which TRN2 instruction encodings
    # require (at most one wait command per instruction).
    nc = bacc.Bacc()
    MMDT = {"bf16": BF16, "f32r": F32R}[mm_dtype]
    emb = nc.declare_dram_parameter("embeds", [NQ, D], F32, isOutput=False)
    centT = nc.declare_dram_parameter("centT", [DC, 128, C], F32, isOutput=False)
    cbrows = nc.declare_dram_parameter("cbrows", [3, C], F32, isOutput=False)
    ones3r = nc.declare_dram_parameter("ones3r", [3, 128], F32, isOutput=False)
    negr2 = nc.declare_dram_parameter("negr2", [128, 1], F32, isOutput=False)
    score_o = nc.declare_dram_parameter("score_out", [NCH, 128], F32, isOutput=True)
    loss_o = nc.declare_dram_parameter("loss_out", [1, 1], F32, isOutput=True)

    emb_r = emb.rearrange("(j p) d -> j p d", p=128)

    with ExitStack() as ctx:
        tc = ctx.enter_context(tile.TileContext(nc))
        singles = ctx.enter_context(tc.tile_pool(name="singles", bufs=1))
        ld = ctx.enter_context(tc.tile_pool(name="ld", bufs=4))
        work = ctx.enter_context(tc.tile_pool(name="work", bufs=3))
        eTp = ctx.enter_context(tc.tile_pool(name="eTp", bufs=3))
        small = ctx.enter_context(tc.tile_pool(name="small", bufs=4))
        pmm = ctx.enter_context(tc.tile_pool(name="pmm", bufs=6, space="PSUM"))
        ptr = ctx.enter_context(tc.tile_pool(name="ptr", bufs=1, space="PSUM"))

        # Replicated constants, staged as f32 then converted on ScalarE so
        # every matmul operand has a single-engine producer (PE LDWEIGHTS
        # carries at most one sync-wait command).
        centT_st = singles.tile([128, DC, C], F32)
        for dc in range(DC):
            nc.sync.dma_start(out=centT_st[:, dc, :], in_=centT[dc])
        centT_sb = singles.tile([128, DC, C], MMDT)
        for dc in range(DC):
            nc.scalar.copy(out=centT_sb[:, dc, :], in_=centT_st[:, dc, :])
        # ||c||^2 as 3 contraction rows: [bf16(cb-512); bf16(residual); ones]
        cb_st = singles.tile([3, C], F32)
        nc.sync.dma_start(out=cb_st, in_=cbrows[:, :])
        cb_sb = singles.tile([3, C], MMDT)
        nc.scalar.copy(out=cb_sb, in_=cb_st)
        # matching stationary rows: [ones; ones; CB_SHIFT] (host-prepared)
        ones3_st = singles.tile([3, 128], F32)
        nc.sync.dma_start(out=ones3_st, in_=ones3r[:, :])
        ones3 = singles.tile([3, 128], MMDT)
        nc.scalar.copy(out=ones3, in_=ones3_st)

        negr2_sb = singles.tile([128, 1], F32)
        nc.sync.dma_start(out=negr2_sb, in_=negr2[:, :])
        ident_f = singles.tile([128, 128], F32)
        make_identity(nc, ident_f)
        ones_col = singles.tile([128, 1], F32)
        nc.vector.memset(ones_col, 1.0)
        s_all = singles.tile([128, NCH], F32)

        for j in range(NCH):
            # cast-DMA (SWDGE): HBM f32 -> SBUF bf16
            e_tile = ld.tile([128, D], MMDT)
            nc.gpsimd.dma_start(out=e_tile, in_=emb_r[j])

            # ||e||^2 per query (ScalarE square + free-dim accumulate)
            esq = work.tile([128, D], MMDT, tag="esq")
            feat = small.tile([128, 1], F32, tag="feat")
            nc.scalar.activation(out=esq, in_=e_tile, func=AF.Square, accum_out=feat)

            # Transpose e [128q, 512d] -> 4x [128d, 128q] on the DMA xbar
            # (SBUF->SBUF, bf16) — keeps the PE free for matmuls
            eT = eTp.tile([128, DC, 128], MMDT)
            for dc in range(DC):
                nc.sync.dma_start_transpose(
                    out=eT[:, dc, :], in_=e_tile[:, dc * 128 : (dc + 1) * 128]
                )

            # psum := ||c||^2 (K=3 reconstruction) - 2 e.c (4 K-chunk dots),
            # then min over the 512 centroids of the chunk on DVE
            minv4 = small.tile([128, NCC], F32, tag="minv4")
            for cc in range(NCC):
                ps = pmm.tile([128, CW], F32, tag="mm")
                nc.tensor.matmul(
                    out=ps,
                    lhsT=ones3,
                    rhs=cb_sb[:, cc * CW : (cc + 1) * CW],
                    start=True,
                    stop=False,
                )
                for dc in range(DC):
                    nc.tensor.matmul(
                        out=ps,
                        lhsT=eT[:, dc, :],
                        rhs=centT_sb[:, dc, cc * CW : (cc + 1) * CW],
                        start=False,
                        stop=(dc == DC - 1),
                    )
                nc.vector.tensor_reduce(
                    out=minv4[:, cc : cc + 1],
                    in_=ps,
                    axis=mybir.AxisListType.X,
                    op=ALU.min,
                )

            minv = small.tile([128, 1], F32, tag="minv")
            nc.vector.tensor_reduce(
                out=minv, in_=minv4, axis=mybir.AxisListType.X, op=ALU.min
            )
            # score = sqrt(min + ||e||^2)
            nc.scalar.activation(
                out=s_all[:, j : j + 1], in_=minv, func=AF.Sqrt, bias=feat, scale=1.0
            )

        # loss partial: sum over all queries of relu(score - r^2)
        junk = singles.tile([128, NCH], F32)
        loss_part = singles.tile([128, 1], F32)
        nc.scalar.activation(
            out=junk, in_=s_all, func=AF.Relu, bias=negr2_sb, accum_out=loss_part
        )
        ps_l = ptr.tile([1, 1], F32, tag="psl", bufs=1)
        nc.tensor.matmul(out=ps_l, lhsT=loss_part, rhs=ones_col, start=True, stop=True)
        loss_sb = small.tile([1, 1], F32, tag="losssb")
        nc.scalar.copy(out=loss_sb, in_=ps_l)
        nc.sync.dma_start(out=loss_o[:, :], in_=loss_sb)

        # scores to [chunk, query] layout so DRAM write is contiguous
        ps_s = ptr.tile([NCH, 128], F32, tag="pss", bufs=1)
        nc.tensor.transpose(out=ps_s, in_=s_all, identity=ident_f)
        score_sb = singles.tile([NCH, 128], F32)
        nc.scalar.copy(out=score_sb, in_=ps_s)
        nc.sync.dma_start(out=score_o[:, :], in_=score_sb)

    nc.finalize()
    return nc


def _prepare_inputs(embeds, centroids, r):
    import ml_dtypes

    embeds = np.ascontiguousarray(np.asarray(embeds), dtype=np.float32)
    centroids = np.ascontiguousarray(np.asarray(centroids), dtype=np.float32)
    r = np.asarray(r, dtype=np.float32)

    centT = np.ascontiguousarray((-2.0 * centroids.T).reshape(DC, 128, C))
    cents = np.sum(centroids.astype(np.float64) ** 2, axis=1).astype(np.float32)
    cb_sh = cents - np.float32(CB_SHIFT)
    cb_hi = cb_sh.astype(ml_dtypes.bfloat16).astype(np.float32)
    cb_lo = cb_sh - cb_hi
    cbrows = np.ascontiguousarray(
        np.stack([cb_hi, cb_lo, np.ones_like(cb_sh)]), dtype=np.float32
    )
    ones3r = np.ones((3, 128), dtype=np.float32)
    ones3r[2, :] = CB_SHIFT
    r2 = np.float32(r[0]) * np.float32(r[0])
    negr2 = np.full((128, 1), -r2, dtype=np.float32)

    emb8 = embeds.reshape(NCORES, NQ, D)
    in_maps = [
        {
            "embeds": emb8[i],
            "centT": centT,
            "cbrows": cbrows,
            "ones3r": ones3r,
            "negr2": negr2,
        }
        for i in range(NCORES)
    ]
    return in_maps


def kernel(embeds, centroids, r):
    global _PROG, LAST_RESULTS
    if _PROG is None:
        _PROG = _build_program()

    in_maps = _prepare_inputs(embeds, centroids, r)
    res = run_bass_kernel_spmd(_PROG, in_maps, list(range(NCORES)), **RUN_KWARGS)
    LAST_RESULTS = res

    score = np.stack(
        [res.results[i]["score_out"].reshape(BLOC, N) for i in range(NCORES)]
    ).reshape(B, N).reshape(B, 1, 64, 64).astype(np.float32)
    loss_sum = float(np.sum([res.results[i]["loss_out"][0, 0] for i in range(NCORES)]))
    loss = np.float32((1.0 / NU) * loss_sum / (B * N))
    return loss, score


# revision 27
# speedup vs baseline: 1.8972x; 1.8972x over previous
"""Trainium2 Bass kernel for batched min-distance retrieval (KNN, K=1).

Computes, for embeds [16,4096,512] and centroids [2048,512]:
    score[b,n] = min_c sqrt(||e_bn||^2 + ||c||^2 - 2 e_bn.c)   -> [16,1,64,64]
    loss = (1/NU) * mean(relu(score - r^2))

Sharding: data-parallel over batch B across 8 cores (2 batches/core);
centroid bank replicated; loss partial-sums combined on host.

Per core, per 128-query chunk:
  - embeds are cast-DMA'd to bf16 and PE-transposed to put the contraction
    dim (d) on partitions,
  - bf16 matmuls compute -2 e.c into PSUM [128q x 512c]; each PSUM
    accumulation group is initialized by a K=3 matmul that reconstructs
    ||c||^2 as bf16(cb-512) + bf16(residual) + 512 (keeps the constant
    term at ~1e-4 absolute error despite bf16 operands),
  - DVE min-reduces each PSUM chunk; ScalarE computes ||e||^2
    (Square+accumulate) and the final sqrt(min + ||e||^2).
Loss: ScalarE Relu+accumulate over all scores, partition-reduced with a
tiny fp32 matmul against ones; host combines the 8 per-core partials.
"""

import numpy as np
from contextlib import ExitStack

import concourse.bass as bass
import concourse.tile as tile
import concourse.mybir as mybir
from concourse import bacc
from concourse.bass_utils import run_bass_kernel_spmd
from concourse.masks import make_identity

F32 = mybir.dt.float32
F32R = mybir.dt.float32r
BF16 = mybir.dt.bfloat16
AF = mybir.ActivationFunctionType
ALU = mybir.AluOpType

B, N, D, C = 16, 4096, 512, 2048
NU = 0.001
NCORES = 8
BLOC = B // NCORES            # batches per core
NQ = BLOC * N                 # 8192 queries per core
NCH = NQ // 128               # 64 query chunks of 128
DC = D // 128                 # 4 contraction chunks
CW = 512                      # centroid tile width (one PSUM bank of fp32)
NCC = C // CW                 # 4 centroid chunks
CB_SHIFT = 512.0              # recenters ||c||^2 (E[||c||^2] = D) for bf16

_PROG = None
LAST_RESULTS = None
RUN_KWARGS = {}  # test-harness hook (e.g. trace=True); empty in production


def _build_program(mm_dtype="bf16"):
    # Bacc (not raw Bass): its compile() pipeline splits multi-wait sync
    # conditions into event semaphores, which TRN2 instruction encodings
    # require (at most one wait command per instruction).
    nc = bacc.Bacc()
    MMDT = {"bf16": BF16, "f32r": F32R}[mm_dtype]
    emb = nc.declare_dram_parameter("embeds", [NQ, D], F32, isOutput=False)
    centT = nc.declare_dram_parameter("centT", [DC, 128, C], F32, isOutput=False)
    cbrows = nc.declare_dram_parameter("cbrows", [3, C], F32, isOutput=False)
    ones3r = nc.declare_dram_parameter("ones3r", [3, 128], F32, isOutput=False)
    negr2 = nc.declare_dram_parameter("negr2", [128, 1], F32, isOutput=False)
    score_o = nc.declare_dram_parameter("score_out", [NCH, 128], F32, isOutput=True)
    loss_o = nc.declare_dram_parameter("loss_out", [1, 1], F32, isOutput=True)

    emb_r = emb.rearrange("(j p) d -> j p d", p=128)

    with ExitStack() as ctx:
        tc = ctx.enter_context(tile.TileContext(nc))
        singles = ctx.enter_context(tc.tile_pool(name="singles", bufs=1))
        ld = ctx.enter_context(tc.tile_pool(name="ld", bufs=4))
        work = ctx.enter_context(tc.tile_pool(name="work", bufs=3))
        eTp = ctx.enter_context(tc.tile_pool(name="eTp", bufs=3))
        small = ctx.enter_context(tc.tile_pool(name="small", bufs=4))
        pmm = ctx.enter_context(tc.tile_pool(name="pmm", bufs=4, space="PSUM"))
        ptr = ctx.enter_context(tc.tile_pool(name="ptr", bufs=1, space="PSUM"))

        # Replicated constants, staged as f32 then converted on ScalarE so
        # every matmul operand has a single-engine producer (PE LDWEIGHTS
        # carries at most one sync-wait command).
        centT_st = singles.tile([128, DC, C], F32)
        for dc in range(DC):
            nc.sync.dma_start(out=centT_st[:, dc, :], in_=centT[dc])
        centT_sb = singles.tile([128, DC, C], MMDT)
        for dc in range(DC):
            nc.scalar.copy(out=centT_sb[:, dc, :], in_=centT_st[:, dc, :])
        # ||c||^2 as 3 contraction rows: [bf16(cb-512); bf16(residual); ones]
        cb_st = singles.tile([3, C], F32)
        nc.sync.dma_start(out=cb_st, in_=cbrows[:, :])
        cb_sb = singles.tile([3, C], MMDT)
        nc.scalar.copy(out=cb_sb, in_=cb_st)
        # matching stationary rows: [ones; ones; CB_SHIFT] (host-prepared)
        ones3_st = singles.tile([3, 128], F32)
        nc.sync.dma_start(out=ones3_st, in_=ones3r[:, :])
        ones3 = singles.tile([3, 128], MMDT)
        nc.scalar.copy(out=ones3, in_=ones3_st)

        negr2_sb = singles.tile([128, 1], F32)
        nc.sync.dma_start(out=negr2_sb, in_=negr2[:, :])
        ident_b = singles.tile([128, 128], MMDT)
        make_identity(nc, ident_b)
        ident_f = singles.tile([128, 128], F32)
        make_identity(nc, ident_f)
        # Dummy transpose: lets PE observe the gpsimd sem once, so the real
        # per-chunk transposes only carry their DMA wait (1-wait LDW limit).
        warm_ps = ptr.tile([128, 128], MMDT, tag="pst", bufs=2)
        nc.tensor.transpose(out=warm_ps, in_=ident_b, identity=ident_b)
        ones_col = singles.tile([128, 1], F32)
        nc.vector.memset(ones_col, 1.0)
        s_all = singles.tile([128, NCH], F32)

        for j in range(NCH):
            # cast-DMA (SWDGE): HBM f32 -> SBUF bf16
            e_tile = ld.tile([128, D], MMDT)
            nc.gpsimd.dma_start(out=e_tile, in_=emb_r[j])

            # ||e||^2 per query (ScalarE square + free-dim accumulate)
            esq = work.tile([128, D], MMDT, tag="esq")
            feat = small.tile([128, 1], F32, tag="feat")
            nc.scalar.activation(out=esq, in_=e_tile, func=AF.Square, accum_out=feat)

            # Transpose e [128q, 512d] -> 4x [128d, 128q] via PE (bf16)
            eT = eTp.tile([128, DC, 128], MMDT)
            for dc in range(DC):
                ps_t = ptr.tile([128, 128], MMDT, tag="pst", bufs=2)
                nc.tensor.transpose(
                    out=ps_t, in_=e_tile[:, dc * 128 : (dc + 1) * 128], identity=ident_b
                )
                nc.vector.tensor_copy(out=eT[:, dc, :], in_=ps_t)

            # psum := ||c||^2 (K=3 reconstruction) - 2 e.c (4 K-chunk dots),
            # then min over the 512 centroids of the chunk on DVE
            minv4 = small.tile([128, NCC], F32, tag="minv4")
            for cc in range(NCC):
                ps = pmm.tile([128, CW], F32, tag="mm")
                nc.tensor.matmul(
                    out=ps,
                    lhsT=ones3,
                    rhs=cb_sb[:, cc * CW : (cc + 1) * CW],
                    start=True,
                    stop=False,
                )
                for dc in range(DC):
                    nc.tensor.matmul(
                        out=ps,
                        lhsT=eT[:, dc, :],
                        rhs=centT_sb[:, dc, cc * CW : (cc + 1) * CW],
                        start=False,
                        stop=(dc == DC - 1),
                    )
                nc.vector.tensor_reduce(
                    out=minv4[:, cc : cc + 1],
                    in_=ps,
                    axis=mybir.AxisListType.X,
                    op=ALU.min,
                )

            minv = small.tile([128, 1], F32, tag="minv")
            nc.vector.tensor_reduce(
                out=minv, in_=minv4, axis=mybir.AxisListType.X, op=ALU.min
            )
            # score = sqrt(min + ||e||^2)
            nc.scalar.activation(
                out=s_all[:, j : j + 1], in_=minv, func=AF.Sqrt, bias=feat, scale=1.0
            )

        # loss partial: sum over all queries of relu(score - r^2)
        junk = singles.tile([128, NCH], F32)
        loss_part = singles.tile([128, 1], F32)
        nc.scalar.activation(
            out=junk, in_=s_all, func=AF.Relu, bias=negr2_sb, accum_out=loss_part
        )
        ps_l = ptr.tile([1, 1], F32, tag="psl", bufs=1)
        nc.tensor.matmul(out=ps_l, lhsT=loss_part, rhs=ones_col, start=True, stop=True)
        loss_sb = small.tile([1, 1], F32, tag="losssb")
        nc.scalar.copy(out=loss_sb, in_=ps_l)
        nc.sync.dma_start(out=loss_o[:, :], in_=loss_sb)

        # scores to [chunk, query] layout so DRAM write is contiguous
        ps_s = ptr.tile([NCH, 128], F32, tag="pss", bufs=1)
        nc.tensor.transpose(out=ps_s, in_=s_all, identity=ident_f)
        score_sb = singles.tile([NCH, 128], F32)
        nc.scalar.copy(out=score_sb, in_=ps_s)
        nc.sync.dma_start(out=score_o[:, :], in_=score_sb)

    nc.finalize()
    return nc


def _prepare_inputs(embeds, centroids, r):
    import ml_dtypes

    embeds = np.ascontiguousarray(np.asarray(embeds), dtype=np.float32)
    centroids = np.ascontiguousarray(np.asarray(centroids), dtype=np.float32)
    r = np.asarray(r, dtype=np.float32)

    centT = np.ascontiguousarray((-2.0 * centroids.T).reshape(DC, 128, C))
    cents = np.sum(centroids.astype(np.float64) ** 2, axis=1).astype(np.float32)
    cb_sh = cents - np.float32(CB_SHIFT)
    cb_hi = cb_sh.astype(ml_dtypes.bfloat16).astype(np.float32)
    cb_lo = cb_sh - cb_hi
    cbrows = np.ascontiguousarray(
        np.stack([cb_hi, cb_lo, np.ones_like(cb_sh)]), dtype=np.float32
    )
    ones3r = np.ones((3, 128), dtype=np.float32)
    ones3r[2, :] = CB_SHIFT
    r2 = np.float32(r[0]) * np.float32(r[0])
    negr2 = np.full((128, 1), -r2, dtype=np.float32)

    emb8 = embeds.reshape(NCORES, NQ, D)
    in_maps = [
        {
            "embeds": emb8[i],
            "centT": centT,
            "cbrows": cbrows,
            "ones3r": ones3r,
            "negr2": negr2,
        }
        for i in range(NCORES)
    ]
    return in_maps


def kernel(embeds, centroids, r):
    global _PROG, LAST_RESULTS
    if _PROG is None:
        _PROG = _build_program()

    in_maps = _prepare_inputs(embeds, centroids, r)
    res = run_bass_kernel_spmd(_PROG, in_maps, list(range(NCORES)), **RUN_KWARGS)
    LAST_RESULTS = res

    score = np.stack(
        [res.results[i]["score_out"].reshape(BLOC, N) for i in range(NCORES)]
    ).reshape(B, N).reshape(B, 1, 64, 64).astype(np.float32)
    loss_sum = float(np.sum([res.results[i]["loss_out"][0, 0] for i in range(NCORES)]))
    loss = np.float32((1.0 / NU) * loss_sum / (B * N))
    return loss, score


# revision 28
# speedup vs baseline: 1.9084x; 1.0059x over previous
"""Trainium2 Bass kernel for batched min-distance retrieval (KNN, K=1).

Computes, for embeds [16,4096,512] and centroids [2048,512]:
    score[b,n] = min_c sqrt(||e_bn||^2 + ||c||^2 - 2 e_bn.c)   -> [16,1,64,64]
    loss = (1/NU) * mean(relu(score - r^2))

Sharding: data-parallel over batch B across 8 cores (2 batches/core);
centroid bank replicated; loss partial-sums combined on host.

Per core, per 128-query chunk:
  - embeds are cast-DMA'd to bf16 and PE-transposed to put the contraction
    dim (d) on partitions,
  - bf16 matmuls compute -2 e.c into PSUM [128q x 512c]; each PSUM
    accumulation group is initialized by a K=3 matmul that reconstructs
    ||c||^2 as bf16(cb-512) + bf16(residual) + 512 (keeps the constant
    term at ~1e-4 absolute error despite bf16 operands),
  - DVE min-reduces each PSUM chunk; ScalarE computes ||e||^2
    (Square+accumulate) and the final sqrt(min + ||e||^2).
Loss: ScalarE Relu+accumulate over all scores, partition-reduced with a
tiny fp32 matmul against ones; host combines the 8 per-core partials.
"""

import numpy as np
from contextlib import ExitStack

import concourse.bass as bass
import concourse.tile as tile
import concourse.mybir as mybir
from concourse import bacc
from concourse.bass_utils import run_bass_kernel_spmd
from concourse.masks import make_identity

F32 = mybir.dt.float32
F32R = mybir.dt.float32r
BF16 = mybir.dt.bfloat16
AF = mybir.ActivationFunctionType
ALU = mybir.AluOpType

B, N, D, C = 16, 4096, 512, 2048
NU = 0.001
NCORES = 8
BLOC = B // NCORES            # batches per core
NQ = BLOC * N                 # 8192 queries per core
NCH = NQ // 128               # 64 query chunks of 128
DC = D // 128                 # 4 contraction chunks
CW = 512                      # centroid tile width (one PSUM bank of fp32)
NCC = C // CW                 # 4 centroid chunks
CB_SHIFT = 512.0              # recenters ||c||^2 (E[||c||^2] = D) for bf16

_PROG = None
LAST_RESULTS = None
RUN_KWARGS = {}  # test-harness hook (e.g. trace=True); empty in production


def _build_program(mm_dtype="bf16"):
    # Bacc (not raw Bass): its compile() pipeline splits multi-wait sync
    # conditions into event semaphores, which TRN2 instruction encodings
    # require (at most one wait command per instruction).
    nc = bacc.Bacc()
    MMDT = {"bf16": BF16, "f32r": F32R}[mm_dtype]
    emb = nc.declare_dram_parameter("embeds", [NQ, D], F32, isOutput=False)
    centT = nc.declare_dram_parameter("centT", [DC, 128, C], F32, isOutput=False)
    cbrows = nc.declare_dram_parameter("cbrows", [3, C], F32, isOutput=False)
    ones3r = nc.declare_dram_parameter("ones3r", [3, 128], F32, isOutput=False)
    identr = nc.declare_dram_parameter("identr", [128, 128], F32, isOutput=False)
    negr2 = nc.declare_dram_parameter("negr2", [128, 1], F32, isOutput=False)
    score_o = nc.declare_dram_parameter("score_out", [NCH, 128], F32, isOutput=True)
    loss_o = nc.declare_dram_parameter("loss_out", [1, 1], F32, isOutput=True)

    emb_r = emb.rearrange("(j p) d -> j p d", p=128)

    with ExitStack() as ctx:
        tc = ctx.enter_context(tile.TileContext(nc))
        singles = ctx.enter_context(tc.tile_pool(name="singles", bufs=1))
        ld = ctx.enter_context(tc.tile_pool(name="ld", bufs=6))
        work = ctx.enter_context(tc.tile_pool(name="work", bufs=3))
        eTp = ctx.enter_context(tc.tile_pool(name="eTp", bufs=4))
        small = ctx.enter_context(tc.tile_pool(name="small", bufs=4))
        pmm = ctx.enter_context(tc.tile_pool(name="pmm", bufs=4, space="PSUM"))
        ptr = ctx.enter_context(tc.tile_pool(name="ptr", bufs=1, space="PSUM"))

        # Replicated constants, staged as f32 then converted on ScalarE so
        # every matmul operand has a single-engine producer (PE LDWEIGHTS
        # carries at most one sync-wait command).
        centT_st = singles.tile([128, DC, C], F32)
        for dc in range(DC):
            nc.sync.dma_start(out=centT_st[:, dc, :], in_=centT[dc])
        centT_sb = singles.tile([128, DC, C], MMDT)
        for dc in range(DC):
            nc.scalar.copy(out=centT_sb[:, dc, :], in_=centT_st[:, dc, :])
        # ||c||^2 as 3 contraction rows: [bf16(cb-512); bf16(residual); ones]
        cb_st = singles.tile([3, C], F32)
        nc.sync.dma_start(out=cb_st, in_=cbrows[:, :])
        cb_sb = singles.tile([3, C], MMDT)
        nc.scalar.copy(out=cb_sb, in_=cb_st)
        # matching stationary rows: [ones; ones; CB_SHIFT] (host-prepared)
        ones3_st = singles.tile([3, 128], F32)
        nc.sync.dma_start(out=ones3_st, in_=ones3r[:, :])
        ones3 = singles.tile([3, 128], MMDT)
        nc.scalar.copy(out=ones3, in_=ones3_st)

        negr2_sb = singles.tile([128, 1], F32)
        nc.sync.dma_start(out=negr2_sb, in_=negr2[:, :])
        ident_f = singles.tile([128, 128], F32)
        nc.sync.dma_start(out=ident_f, in_=identr[:, :])
        ident_b = singles.tile([128, 128], MMDT)
        nc.scalar.copy(out=ident_b, in_=ident_f)
        # Dummy transpose: lets PE observe the gpsimd sem once, so the real
        # per-chunk transposes only carry their DMA wait (1-wait LDW limit).
        warm_ps = ptr.tile([128, 128], MMDT, tag="pst", bufs=2)
        nc.tensor.transpose(out=warm_ps, in_=ident_b, identity=ident_b)
        ones_col = singles.tile([128, 1], F32)
        nc.vector.memset(ones_col, 1.0)
        s_all = singles.tile([128, NCH], F32)

        for j in range(NCH):
            # cast-DMA (SWDGE): HBM f32 -> SBUF bf16
            e_tile = ld.tile([128, D], MMDT)
            nc.gpsimd.dma_start(out=e_tile, in_=emb_r[j])

            # ||e||^2 per query (ScalarE square + free-dim accumulate)
            esq = work.tile([128, D], MMDT, tag="esq")
            feat = small.tile([128, 1], F32, tag="feat")
            nc.scalar.activation(out=esq, in_=e_tile, func=AF.Square, accum_out=feat)

            # Transpose e [128q, 512d] -> 4x [128d, 128q] via PE (bf16)
            eT = eTp.tile([128, DC, 128], MMDT)
            for dc in range(DC):
                ps_t = ptr.tile([128, 128], MMDT, tag="pst", bufs=2)
                nc.tensor.transpose(
                    out=ps_t, in_=e_tile[:, dc * 128 : (dc + 1) * 128], identity=ident_b
                )
                nc.vector.tensor_copy(out=eT[:, dc, :], in_=ps_t)

            # psum := ||c||^2 (K=3 reconstruction) - 2 e.c (4 K-chunk dots),
            # then min over the 512 centroids of the chunk on DVE
            minv4 = small.tile([128, NCC], F32, tag="minv4")
            for cc in range(NCC):
                ps = pmm.tile([128, CW], F32, tag="mm")
                nc.tensor.matmul(
                    out=ps,
                    lhsT=ones3,
                    rhs=cb_sb[:, cc * CW : (cc + 1) * CW],
                    start=True,
                    stop=False,
                )
                for dc in range(DC):
                    nc.tensor.matmul(
                        out=ps,
                        lhsT=eT[:, dc, :],
                        rhs=centT_sb[:, dc, cc * CW : (cc + 1) * CW],
                        start=False,
                        stop=(dc == DC - 1),
                    )
                nc.vector.tensor_reduce(
                    out=minv4[:, cc : cc + 1],
                    in_=ps,
                    axis=mybir.AxisListType.X,
                    op=ALU.min,
                )

            minv = small.tile([128, 1], F32, tag="minv")
            nc.vector.tensor_reduce(
                out=minv, in_=minv4, axis=mybir.AxisListType.X, op=ALU.min
            )
            # score = sqrt(min + ||e||^2)
            nc.scalar.activation(
                out=s_all[:, j : j + 1], in_=minv, func=AF.Sqrt, bias=feat, scale=1.0
            )

        # scores to [chunk, query] layout so DRAM write is contiguous
        ps_s = ptr.tile([NCH, 128], F32, tag="pss", bufs=1)
        nc.tensor.transpose(out=ps_s, in_=s_all, identity=ident_f)
        score_sb = singles.tile([NCH, 128], F32)
        nc.vector.tensor_copy(out=score_sb, in_=ps_s)
        nc.sync.dma_start(out=score_o[:, :], in_=score_sb)

        # loss partial: sum over all queries of relu(score - r^2)
        junk = singles.tile([128, NCH], F32)
        loss_part = singles.tile([128, 1], F32)
        nc.scalar.activation(
            out=junk, in_=s_all, func=AF.Relu, bias=negr2_sb, accum_out=loss_part
        )
        ps_l = ptr.tile([1, 1], F32, tag="psl", bufs=1)
        nc.tensor.matmul(out=ps_l, lhsT=loss_part, rhs=ones_col, start=True, stop=True)
        loss_sb = small.tile([1, 1], F32, tag="losssb")
        nc.scalar.copy(out=loss_sb, in_=ps_l)
        nc.sync.dma_start(out=loss_o[:, :], in_=loss_sb)

    nc.finalize()
    return nc


def _prepare_inputs(embeds, centroids, r):
    import ml_dtypes

    embeds = np.ascontiguousarray(np.asarray(embeds), dtype=np.float32)
    centroids = np.ascontiguousarray(np.asarray(centroids), dtype=np.float32)
    r = np.asarray(r, dtype=np.float32)

    centT = np.ascontiguousarray((-2.0 * centroids.T).reshape(DC, 128, C))
    cents = np.sum(centroids.astype(np.float64) ** 2, axis=1).astype(np.float32)
    cb_sh = cents - np.float32(CB_SHIFT)
    cb_hi = cb_sh.astype(ml_dtypes.bfloat16).astype(np.float32)
    cb_lo = cb_sh - cb_hi
    cbrows = np.ascontiguousarray(
        np.stack([cb_hi, cb_lo, np.ones_like(cb_sh)]), dtype=np.float32
    )
    ones3r = np.ones((3, 128), dtype=np.float32)
    ones3r[2, :] = CB_SHIFT
    identr = np.eye(128, dtype=np.float32)
    r2 = np.float32(r[0]) * np.float32(r[0])
    negr2 = np.full((128, 1), -r2, dtype=np.float32)

    emb8 = embeds.reshape(NCORES, NQ, D)
    in_maps = [
        {
            "embeds": emb8[i],
            "centT": centT,
            "cbrows": cbrows,
            "ones3r": ones3r,
            "identr": identr,
            "negr2": negr2,
        }
        for i in range(NCORES)
    ]
    return in_maps


def kernel(embeds, centroids, r):
    global _PROG, LAST_RESULTS
    if _PROG is None:
        _PROG = _build_program()

    in_maps = _prepare_inputs(embeds, centroids, r)
    res = run_bass_kernel_spmd(_PROG, in_maps, list(range(NCORES)), **RUN_KWARGS)
    LAST_RESULTS = res

    score = np.stack(
        [res.results[i]["score_out"].reshape(BLOC, N) for i in range(NCORES)]
    ).reshape(B, N).reshape(B, 1, 64, 64).astype(np.float32)
    loss_sum = float(np.sum([res.results[i]["loss_out"][0, 0] for i in range(NCORES)]))
    loss = np.float32((1.0 / NU) * loss_sum / (B * N))
    return loss, score


# revision 30
# speedup vs baseline: 1.9509x; 1.0222x over previous
"""Trainium2 Bass kernel for batched min-distance retrieval (KNN, K=1).

Computes, for embeds [16,4096,512] and centroids [2048,512]:
    score[b,n] = min_c sqrt(||e_bn||^2 + ||c||^2 - 2 e_bn.c)   -> [16,1,64,64]
    loss = (1/NU) * mean(relu(score - r^2))

Sharding: data-parallel over batch B across 8 cores (2 batches/core);
centroid bank replicated; loss partial-sums combined on host.

Per core, per 128-query chunk:
  - embeds are cast-DMA'd to bf16 and PE-transposed to put the contraction
    dim (d) on partitions,
  - bf16 matmuls compute -2 e.c into PSUM [128q x 512c]; each PSUM
    accumulation group is initialized by a K=3 matmul that reconstructs
    ||c||^2 as bf16(cb-512) + bf16(residual) + 512 (keeps the constant
    term at ~1e-4 absolute error despite bf16 operands),
  - DVE min-reduces each PSUM chunk; ScalarE computes ||e||^2
    (Square+accumulate) and the final sqrt(min + ||e||^2).
Loss: ScalarE Relu+accumulate over all scores, partition-reduced with a
tiny fp32 matmul against ones; host combines the 8 per-core partials.
"""

import numpy as np
from contextlib import ExitStack

import concourse.bass as bass
import concourse.tile as tile
import concourse.mybir as mybir
from concourse import bacc
from concourse.bass_utils import run_bass_kernel_spmd
from concourse.masks import make_identity

F32 = mybir.dt.float32
F32R = mybir.dt.float32r
BF16 = mybir.dt.bfloat16
AF = mybir.ActivationFunctionType
ALU = mybir.AluOpType

B, N, D, C = 16, 4096, 512, 2048
NU = 0.001
NCORES = 8
BLOC = B // NCORES            # batches per core
NQ = BLOC * N                 # 8192 queries per core
NCH = NQ // 128               # 64 query chunks of 128
DC = D // 128                 # 4 contraction chunks
CW = 512                      # centroid tile width (one PSUM bank of fp32)
NCC = C // CW                 # 4 centroid chunks
CB_SHIFT = 512.0              # recenters ||c||^2 (E[||c||^2] = D) for bf16

_PROG = None
LAST_RESULTS = None
RUN_KWARGS = {}  # test-harness hook (e.g. trace=True); empty in production


def _build_program(mm_dtype="bf16"):
    # Bacc (not raw Bass): its compile() pipeline splits multi-wait sync
    # conditions into event semaphores, which TRN2 instruction encodings
    # require (at most one wait command per instruction).
    nc = bacc.Bacc()
    MMDT = {"bf16": BF16, "f32r": F32R}[mm_dtype]
    emb = nc.declare_dram_parameter("embeds", [NQ, D], F32, isOutput=False)
    centT = nc.declare_dram_parameter("centT", [NCC, DC, 128, CW], F32, isOutput=False)
    cbrows = nc.declare_dram_parameter("cbrows", [3, C], F32, isOutput=False)
    ones3r = nc.declare_dram_parameter("ones3r", [3, 128], F32, isOutput=False)
    identr = nc.declare_dram_parameter("identr", [128, 128], F32, isOutput=False)
    negr2 = nc.declare_dram_parameter("negr2", [128, 1], F32, isOutput=False)
    score_o = nc.declare_dram_parameter("score_out", [NCH, 128], F32, isOutput=True)
    loss_o = nc.declare_dram_parameter("loss_out", [2, 1], F32, isOutput=True)

    emb_r = emb.rearrange("(j p) d -> j p d", p=128)

    with ExitStack() as ctx:
        tc = ctx.enter_context(tile.TileContext(nc))
        singles = ctx.enter_context(tc.tile_pool(name="singles", bufs=1))
        ld = ctx.enter_context(tc.tile_pool(name="ld", bufs=6))
        work = ctx.enter_context(tc.tile_pool(name="work", bufs=3))
        eTp = ctx.enter_context(tc.tile_pool(name="eTp", bufs=4))
        small = ctx.enter_context(tc.tile_pool(name="small", bufs=4))
        pmm = ctx.enter_context(tc.tile_pool(name="pmm", bufs=4, space="PSUM"))
        ptr = ctx.enter_context(tc.tile_pool(name="ptr", bufs=1, space="PSUM"))

        # Replicated constants, cast f32 -> bf16 during the DMA itself
        # (SWDGE) so neither ScalarE nor the PE wait on staging copies.
        # centT arrives cc-major so the first 512-centroid slab is ready
        # after ~1 MB of traffic.
        ident_b = singles.tile([128, 128], MMDT)
        nc.gpsimd.dma_start(out=ident_b, in_=identr[:, :])
        ones3 = singles.tile([3, 128], MMDT)
        nc.gpsimd.dma_start(out=ones3, in_=ones3r[:, :])
        cb_sb = singles.tile([3, C], MMDT)
        nc.gpsimd.dma_start(out=cb_sb, in_=cbrows[:, :])
        centT_sb = singles.tile([128, DC, C], MMDT)
        for cc in range(NCC):
            nc.gpsimd.dma_start(
                out=centT_sb[:, :, cc * CW : (cc + 1) * CW],
                in_=centT[cc].rearrange("dc p w -> p dc w"),
            )
        negr2_sb = singles.tile([128, 1], F32)
        nc.sync.dma_start(out=negr2_sb, in_=negr2[:, :])
        ident_f = singles.tile([128, 128], F32)
        nc.sync.dma_start(out=ident_f, in_=identr[:, :])
        ones_col = singles.tile([128, 1], F32)
        nc.vector.memset(ones_col, 1.0)
        s_all = singles.tile([128, NCH], F32)
        # tail tiles, written half-way and at the end
        score_sb = singles.tile([NCH, 128], F32)
        junk = singles.tile([128, NCH], F32)
        loss_ab = singles.tile([128, 2], F32)
        HALF = NCH // 2

        for j in range(NCH):
            # cast-DMA (SWDGE): HBM f32 -> SBUF bf16
            e_tile = ld.tile([128, D], MMDT)
            nc.gpsimd.dma_start(out=e_tile, in_=emb_r[j])

            # ||e||^2 per query (ScalarE square + free-dim accumulate)
            esq = work.tile([128, D], MMDT, tag="esq")
            feat = small.tile([128, 1], F32, tag="feat")
            nc.scalar.activation(out=esq, in_=e_tile, func=AF.Square, accum_out=feat)

            # Transpose e [128q, 512d] -> 4x [128d, 128q] via PE (bf16)
            eT = eTp.tile([128, DC, 128], MMDT)
            for dc in range(DC):
                ps_t = ptr.tile([128, 128], MMDT, tag="pst", bufs=2)
                nc.tensor.transpose(
                    out=ps_t, in_=e_tile[:, dc * 128 : (dc + 1) * 128], identity=ident_b
                )
                nc.vector.tensor_copy(out=eT[:, dc, :], in_=ps_t)

            # psum := ||c||^2 (K=3 reconstruction) - 2 e.c (4 K-chunk dots),
            # then min over the 512 centroids of the chunk on DVE
            minv4 = small.tile([128, NCC], F32, tag="minv4")
            for cc in range(NCC):
                ps = pmm.tile([128, CW], F32, tag="mm")
                nc.tensor.matmul(
                    out=ps,
                    lhsT=ones3,
                    rhs=cb_sb[:, cc * CW : (cc + 1) * CW],
                    start=True,
                    stop=False,
                )
                for dc in range(DC):
                    nc.tensor.matmul(
                        out=ps,
                        lhsT=eT[:, dc, :],
                        rhs=centT_sb[:, dc, cc * CW : (cc + 1) * CW],
                        start=False,
                        stop=(dc == DC - 1),
                    )
                nc.vector.tensor_reduce(
                    out=minv4[:, cc : cc + 1],
                    in_=ps,
                    axis=mybir.AxisListType.X,
                    op=ALU.min,
                )

            minv = small.tile([128, 1], F32, tag="minv")
            nc.vector.tensor_reduce(
                out=minv, in_=minv4, axis=mybir.AxisListType.X, op=ALU.min
            )
            # score = sqrt(min + ||e||^2)
            nc.scalar.activation(
                out=s_all[:, j : j + 1], in_=minv, func=AF.Sqrt, bias=feat, scale=1.0
            )

            if j in (HALF - 1, NCH - 1):
                h = 0 if j == HALF - 1 else 1
                sl = slice(h * HALF, (h + 1) * HALF)
                ps_sh = ptr.tile([HALF, 128], F32, tag="pss", bufs=1)
                nc.tensor.transpose(out=ps_sh, in_=s_all[:, sl], identity=ident_f)
                nc.vector.tensor_copy(
                    out=score_sb[h * HALF : (h + 1) * HALF, :],
                    in_=ps_sh,
                )
                nc.sync.dma_start(
                    out=score_o[h * HALF : (h + 1) * HALF, :],
                    in_=score_sb[h * HALF : (h + 1) * HALF, :],
                )
                nc.scalar.activation(
                    out=junk[:, sl],
                    in_=s_all[:, sl],
                    func=AF.Relu,
                    bias=negr2_sb,
                    accum_out=loss_ab[:, h : h + 1],
                )

        # loss: partition-reduce the two half partials with a tiny matmul
        ps_l = ptr.tile([2, 1], F32, tag="psl", bufs=1)
        nc.tensor.matmul(out=ps_l, lhsT=loss_ab, rhs=ones_col, start=True, stop=True)
        loss_sb = small.tile([2, 1], F32, tag="losssb")
        nc.scalar.copy(out=loss_sb, in_=ps_l)
        nc.sync.dma_start(out=loss_o[:, :], in_=loss_sb)

    nc.finalize()
    return nc


def _prepare_inputs(embeds, centroids, r):
    import ml_dtypes

    embeds = np.ascontiguousarray(np.asarray(embeds), dtype=np.float32)
    centroids = np.ascontiguousarray(np.asarray(centroids), dtype=np.float32)
    r = np.asarray(r, dtype=np.float32)

    centT = np.ascontiguousarray(
        (-2.0 * centroids.T).reshape(DC, 128, NCC, CW).transpose(2, 0, 1, 3)
    )
    cents = np.sum(centroids.astype(np.float64) ** 2, axis=1).astype(np.float32)
    cb_sh = cents - np.float32(CB_SHIFT)
    cb_hi = cb_sh.astype(ml_dtypes.bfloat16).astype(np.float32)
    cb_lo = cb_sh - cb_hi
    cbrows = np.ascontiguousarray(
        np.stack([cb_hi, cb_lo, np.ones_like(cb_sh)]), dtype=np.float32
    )
    ones3r = np.ones((3, 128), dtype=np.float32)
    ones3r[2, :] = CB_SHIFT
    identr = np.eye(128, dtype=np.float32)
    r2 = np.float32(r[0]) * np.float32(r[0])
    negr2 = np.full((128, 1), -r2, dtype=np.float32)

    emb8 = embeds.reshape(NCORES, NQ, D)
    in_maps = [
        {
            "embeds": emb8[i],
            "centT": centT,
            "cbrows": cbrows,
            "ones3r": ones3r,
            "identr": identr,
            "negr2": negr2,
        }
        for i in range(NCORES)
    ]
    return in_maps


def kernel(embeds, centroids, r):
    global _PROG, LAST_RESULTS
    if _PROG is None:
        _PROG = _build_program()

    in_maps = _prepare_inputs(embeds, centroids, r)
    res = run_bass_kernel_spmd(_PROG, in_maps, list(range(NCORES)), **RUN_KWARGS)
    LAST_RESULTS = res

    score = np.stack(
        [res.results[i]["score_out"].reshape(BLOC, N) for i in range(NCORES)]
    ).reshape(B, N).reshape(B, 1, 64, 64).astype(np.float32)
    loss_sum = float(np.sum([np.sum(res.results[i]["loss_out"]) for i in range(NCORES)]))
    loss = np.float32((1.0 / NU) * loss_sum / (B * N))
    return loss, score
